# revision 8
# baseline (speedup 1.0000x reference)
# Trainium2 Bass kernel for:
#   q = x @ Wq.T + bq ; k = x @ Wk.T + bk ; v = x @ Wv.T + bv
#   g = sigmoid(x @ Wg.T + bg)
#   out = q * cumsum(k*v, axis=seq) * g
#
# Sharding: tensor-parallel split of the 2048 output features across the 8
# cores (256 features each). All ops are per-feature except the d-contraction
# (each core uses the full x) and the cumsum along seq (handled fully on-core
# per (batch, feature)) -> zero cross-core communication.
#
# v3 over the bf16 baseline (912.5us):
#   - Part of the contraction runs as fp8(e4m3) DoubleRow matmuls: 14 of
#     g's 16 chunks and 2 of v's (numerics sim vs the jax reference —
#     which matched HW to 4 digits on two configs — puts max-err/scale
#     at 1.57e-2 vs the 2e-2 gate; more fp8 anywhere crosses 1.9e-2).
#     Warm DR matmuls stream 256 contraction rows in the same 216ns a
#     bf16 matmul needs for 128 rows (full 2x; LDW hidden).
#   - The fp8 and bf16 partial sums share one PSUM accumulator by
#     pre-scaling x by 32 and Wv/Wg by 4096 (exact powers of 2); the ACT
#     descales (1/32 for k,q; 2^-17 for v,g) in the existing bias-add.
#   - DR->bf16 mode switches cost ~220ns, so the v-DR and g-DR sections
#     are emitted back-to-back (chain order k,q,v,g with v's DR last and
#     g's DR first) -> one switch per m-group instead of two.
#   - fp32 downstream (k,v,q,g,kv,qg,out + out DMA) halves the rounding
#     floor (6.7e-3 -> 3.1e-3), buying the fp8 error budget.
#   - 110 dummy warmup matmuls on a zeroed tile keep the PE busy from the
#     end of the ~7.4us framework init so the HAM clock gate opens
#     (1.2 -> 2.4 GHz) during the DMA-bound prologue, not at t=26us.
#   - W packed m-major ([128, MH, KC, 128]) so the first chain only
#     gates on x(0,0) + Wk's m=0 half; halves stream in consumption
#     order; x8 rides the scalar SWDGE queue in parallel with the big
#     sync-queue stream.
#
# On-core layout is [e, t] (features on partitions, tokens on the free dim):
#   - linears:  psum[e,t] += W_chunk.T @ x_chunk   (fp32 accum)
#   - bias:     ACT activation Identity with per-partition bias + descale
#   - sigmoid:  ACT activation with per-partition bias + descale
#   - cumsum:   DVE tensor_tensor_scan along the free dim (fp32),
#               chained across token (sub)tiles via initial=prev[:, -1:]
#   - qg mul on the Pool engine, kv/out muls on DVE.
# The final unit is processed in 128-token sub-tiles to shorten the
# post-matmul drain chain.

from contextlib import ExitStack

import numpy as np
import ml_dtypes

import concourse.bass as bass  # noqa: F401  (bass types referenced via tile/bacc)
import concourse.tile as tile
from concourse import bacc, mybir
from concourse.bass_utils import run_bass_kernel_spmd

N_CORES = 8
B, S, D = 4, 4096, 2048
E = D // N_CORES  # 256 output features per core
TT = 512          # token tile (free dim of psum)
KC = D // 128     # contraction chunks
NU = S // TT      # token tiles per batch
MH = E // 128     # feature halves (psum groups per linear)
MM_DT = mybir.dt.bfloat16
MM_NP = ml_dtypes.bfloat16
F8_DT = mybir.dt.float8e4
F8_NP = ml_dtypes.float8_e4m3  # TRN fp8e4: max normal 240, matches after clip

# per-chain fp8 contraction chunks (each must be even; fp8 covers the FIRST
# nf8 chunks of that chain's contraction)
NF8 = {"q": 0, "k": 0, "v": 2, "g": 14}
NX8 = max(NF8.values())  # chunks of x kept in fp8
SX = 32.0                # x pre-scale (exact in bf16; uses e4m3 range)
SW = 4096.0              # W pre-scale for chains with fp8 chunks
N_WARM = 140             # dummy warmup matmuls before the real stream


def build_nc(b=B, s=S, d=D, e=E, tt=TT, n_cores=N_CORES):
    kc = KC
    nu = NU
    mh = MH
    f32 = mybir.dt.float32
    names = "qkvg"

    nc = bacc.Bacc(
        "TRN2", target_bir_lowering=False, debug=False, num_devices=n_cores
    )
    # x packed on host (pre-scaled by SX): X5[b, n, p, c, t] = SX*x[b, n*tt+t, c*128+p]
    X5 = nc.dram_tensor(
        "X5", [b, nu, 128, kc, tt], MM_DT, kind="ExternalInput"
    ).ap()
    # fp8 copy of x's first NX8 chunks (same SX scale)
    X8 = nc.dram_tensor(
        "X8", [b, nu, 128, NX8, tt], F8_DT, kind="ExternalInput"
    ).ap()
    # W packed on host, m-major: [p, m, c, e'] = W[core_sl][m*128+e', c*128+p]
    # chains with fp8 chunks ship W (x SW) as a fp8 part + a bf16 part
    Wb = {}
    W8 = {}
    for x_ in names:
        n8 = NF8[x_]
        Wb[x_] = nc.dram_tensor(
            f"W{x_}b", [128, mh, kc - n8, 128], MM_DT, kind="ExternalInput"
        ).ap()
        if n8:
            W8[x_] = nc.dram_tensor(
                f"W{x_}8", [128, mh, n8, 128], F8_DT, kind="ExternalInput"
            ).ap()
    bias = {
        x_: nc.dram_tensor(f"b{x_}", [e], f32, kind="ExternalInput").ap()
        for x_ in names
    }
    outT = nc.dram_tensor("outT", [b, e, s], f32, kind="ExternalOutput").ap()

    add = mybir.AluOpType.add
    bypass = mybir.AluOpType.bypass
    mult = mybir.AluOpType.mult
    sigmoid = mybir.ActivationFunctionType.Sigmoid
    identity = mybir.ActivationFunctionType.Identity
    dr = mybir.MatmulPerfMode.DoubleRow
    descale = {x_: 1.0 / (SX * (SW if NF8[x_] else 1.0)) for x_ in names}

    with tile.TileContext(nc) as tc, ExitStack() as ctx:
        wpool = ctx.enter_context(tc.tile_pool(name="w", bufs=1))
        cpool = ctx.enter_context(tc.tile_pool(name="const", bufs=1))
        xpool = ctx.enter_context(tc.tile_pool(name="x", bufs=3))
        x8pool = ctx.enter_context(tc.tile_pool(name="x8", bufs=3))
        ppool = ctx.enter_context(tc.tile_pool(name="psum", bufs=8, space="PSUM"))
        spool = ctx.enter_context(tc.tile_pool(name="work", bufs=5))
        opool = ctx.enter_context(tc.tile_pool(name="out", bufs=3))
        cspool = ctx.enter_context(tc.tile_pool(name="cs", bufs=6))

        # Biases via the gpsimd SWDGE queue (parallel with the sync stream):
        # [128, mh], col m = bias[m*128:(m+1)*128]
        b_sb = {}
        for x_ in names:
            t_ = cpool.tile([128, mh], f32, tag=f"b{x_}")
            nc.gpsimd.dma_start(out=t_, in_=bias[x_].rearrange("(m p) -> p m", p=128))
            b_sb[x_] = t_

        # PE warmup: dummy matmuls on a zeroed tile keep the PE's HAM
        # activity window busy while the prologue DMAs stream, so the
        # 1.2->2.4 GHz un-throttle fires before the real chains start.
        if N_WARM:
            dz = cpool.tile([128, 128], MM_DT, tag="warmz")
            nc.vector.memset(dz[:], 0.0)
            pd = ppool.tile([128, tt], f32, tag="ps")
            for _ in range(N_WARM):
                nc.tensor.matmul(
                    pd[:, :64], lhsT=dz[:], rhs=dz[:, :64], start=True, stop=True
                )

        wb_sb = {}
        w8_sb = {}
        for x_ in names:
            n8 = NF8[x_]
            t_ = wpool.tile([128, mh, kc - n8, 128], MM_DT, tag=f"w{x_}b")
            wb_sb[x_] = t_
            if n8:
                t8_ = wpool.tile([128, mh, n8, 128], F8_DT, tag=f"w{x_}8")
                w8_sb[x_] = t8_

        def load_x(bi, n, n_dmas=1):
            # bf16 unit tile (sync queue) + fp8 chunk tile (scalar queue);
            # n_dmas>1 splits the bf16 transfer so early chunks land (and
            # unblock matmuls) sooner
            t_ = xpool.tile([128, kc, tt], MM_DT, tag="xt")
            step = kc // n_dmas
            for c0 in range(0, kc, step):
                nc.sync.dma_start(
                    out=t_[:, c0:c0 + step, :], in_=X5[bi][n][:, c0:c0 + step, :]
                )
            t8 = x8pool.tile([128, NX8, tt], F8_DT, tag="x8t")
            nc.scalar.dma_start(out=t8, in_=X8[bi][n])
            return t_, t8

        def load_w_half(x_, m):
            # one m-half of a chain's weights, in chain consumption order
            nc.sync.dma_start(out=wb_sb[x_][:, m], in_=Wb[x_][:, m])
            if NF8[x_]:
                nc.sync.dma_start(out=w8_sb[x_][:, m], in_=W8[x_][:, m])

        # Consumption-ordered prologue on the sync queue. Unit (0,0) is
        # processed as two 256-token half-units, so x(0,0) streams
        # token-half-major: half 0's eighths interleaved with Wk-m0
        # quarters (the first chain's operands), then the remaining m0
        # weight halves in chain order (q, v, g), then x half 1, then the
        # m1 halves, then units (0,1)/(0,2). x8(0,0) rides the scalar
        # queue in parallel, also token-half-major.
        x_first = xpool.tile([128, kc, tt], MM_DT, tag="xt")
        x8_first = x8pool.tile([128, NX8, tt], F8_DT, tag="x8t")
        ht = tt // 2
        nc.scalar.dma_start(out=x8_first[:, :, :ht], in_=X8[0][0][:, :, :ht])
        nc.scalar.dma_start(out=x8_first[:, :, ht:], in_=X8[0][0][:, :, ht:])
        for h in range(kc // 2):
            nc.sync.dma_start(
                out=x_first[:, 2 * h:2 * h + 2, :ht],
                in_=X5[0][0][:, 2 * h:2 * h + 2, :ht],
            )
            if h < 4:
                nc.sync.dma_start(
                    out=wb_sb["k"][:, 0, 4 * h:4 * h + 4, :],
                    in_=Wb["k"][:, 0, 4 * h:4 * h + 4, :],
                )
        for x_ in "qvg":
            load_w_half(x_, 0)
        nc.sync.dma_start(out=x_first[:, :, ht:], in_=X5[0][0][:, :, ht:])
        for x_ in "kqvg":
            load_w_half(x_, 1)

        def emit_unit(bi, n, xt, x8t, cs_prev, first_of_seq, vec_sub,
                      toff=0, twid=None):
            """Matmul chains for tokens [toff, toff+twid) of unit (bi, n);
            the ACT/DVE chain runs on vec_sub-wide psum slices (narrow for
            the final unit so the post-matmul drain is short). twid<tt is
            used for the DMA-gated first unit so chains start on a half
            tile's worth of data."""
            tw_ = twid if twid is not None else tt
            tsl = slice(toff, toff + tw_)
            # chain order k,q,v,g: k early for the kv/scan chain, and v's
            # trailing DR section lands adjacent to g's leading one so the
            # DR->bf16 mode switch is paid once per m-group
            ps = {}
            for m in range(mh):
                for x_ in "kq":
                    p_ = ppool.tile([128, tt], f32, tag="ps")
                    for c in range(kc):
                        nc.tensor.matmul(
                            p_[:, :tw_],
                            lhsT=wb_sb[x_][:, m, c, :],
                            rhs=xt[:, c, tsl],
                            start=(c == 0),
                            stop=(c == kc - 1),
                        )
                    ps[x_, m] = p_
                # v: bf16 chunks [nv8, kc) first, then DR pairs [0, nv8)
                nv8 = NF8["v"]
                p_ = ppool.tile([128, tt], f32, tag="ps")
                for i in range(kc - nv8):
                    nc.tensor.matmul(
                        p_[:, :tw_],
                        lhsT=wb_sb["v"][:, m, i, :],
                        rhs=xt[:, nv8 + i, tsl],
                        start=(i == 0),
                        stop=False,
                    )
                for j in range(nv8 // 2):
                    nc.tensor.matmul(
                        p_[:, :tw_],
                        lhsT=w8_sb["v"][:, m, 2 * j:2 * j + 2, :],
                        rhs=x8t[:, 2 * j:2 * j + 2, tsl],
                        start=False,
                        stop=(j == nv8 // 2 - 1),
                        perf_mode=dr,
                    )
                ps["v", m] = p_
                # g: DR pairs [0, ng8) first, then bf16 chunks [ng8, kc)
                ng8 = NF8["g"]
                p_ = ppool.tile([128, tt], f32, tag="ps")
                for j in range(ng8 // 2):
                    nc.tensor.matmul(
                        p_[:, :tw_],
                        lhsT=w8_sb["g"][:, m, 2 * j:2 * j + 2, :],
                        rhs=x8t[:, 2 * j:2 * j + 2, tsl],
                        start=(j == 0),
                        stop=False,
                        perf_mode=dr,
                    )
                for i in range(kc - ng8):
                    nc.tensor.matmul(
                        p_[:, :tw_],
                        lhsT=wb_sb["g"][:, m, i, :],
                        rhs=xt[:, ng8 + i, tsl],
                        start=False,
                        stop=(i == kc - ng8 - 1),
                    )
                ps["g", m] = p_

            for m in range(mh):
                # pass 1: k/v bias-adds (+descale), kv, scan per sub-slice
                # (two-pass so q/g ACT ops never block later slices' k/v in
                # the queue)
                cs_list = []
                for t0 in range(0, tw_, vec_sub):
                    tw = vec_sub
                    sl = slice(t0, t0 + tw)
                    k_sb = spool.tile([128, tw], f32, tag="k")
                    nc.scalar.activation(
                        k_sb[:], ps["k", m][:, sl], identity,
                        bias=b_sb["k"][:, m:m + 1], scale=descale["k"],
                    )
                    v_sb = spool.tile([128, tw], f32, tag="v")
                    nc.scalar.activation(
                        v_sb[:], ps["v", m][:, sl], identity,
                        bias=b_sb["v"][:, m:m + 1], scale=descale["v"],
                    )
                    kv = spool.tile([128, tw], f32, tag="kv")
                    nc.vector.tensor_tensor(kv[:], k_sb[:], v_sb[:], mult)
                    cs = cspool.tile([128, tw], f32, tag="cs")
                    init = (0.0 if first_of_seq and t0 == 0
                            else cs_prev[m][:, -1:])
                    nc.vector.tensor_tensor_scan(
                        cs[:], kv[:], kv[:], init, op0=add, op1=bypass
                    )
                    cs_prev[m] = cs
                    cs_list.append(cs)
                # pass 2: q/g bias-adds, qg, output — sub-slices write into
                # one full-width tile so each m issues a single out-DMA
                o_sb = opool.tile([128, tw_], f32, tag="o")
                q_list = []
                for t0 in range(0, tw_, vec_sub):
                    sl = slice(t0, t0 + vec_sub)
                    q_sb = spool.tile([128, vec_sub], f32, tag="q")
                    nc.scalar.activation(
                        q_sb[:], ps["q", m][:, sl], identity,
                        bias=b_sb["q"][:, m:m + 1], scale=descale["q"],
                    )
                    q_list.append(q_sb)
                for i, t0 in enumerate(range(0, tw_, vec_sub)):
                    sl = slice(t0, t0 + vec_sub)
                    g_sb = spool.tile([128, vec_sub], f32, tag="g")
                    nc.scalar.activation(
                        g_sb[:], ps["g", m][:, sl], sigmoid,
                        bias=b_sb["g"][:, m:m + 1], scale=descale["g"],
                    )
                    qg = spool.tile([128, vec_sub], f32, tag="qg")
                    nc.gpsimd.tensor_tensor(qg[:], q_list[i][:], g_sb[:], mult)
                    nc.vector.tensor_tensor(o_sb[:, sl], qg[:], cs_list[i][:], mult)
                t_base = n * tt + toff
                nc.sync.dma_start(
                    out=outT[bi][m * 128:(m + 1) * 128, t_base:t_base + tw_],
                    in_=o_sb[:],
                )

        for bi in range(b):
            cs_prev = [None] * mh
            for n in range(nu):
                if bi == 0 and n == 0:
                    # DMA-gated start: two half-units so the first chains
                    # only wait on half of x(0,0)
                    ht_ = tt // 2
                    emit_unit(0, 0, x_first, x8_first, cs_prev,
                              first_of_seq=True, vec_sub=ht_, toff=0,
                              twid=ht_)
                    emit_unit(0, 0, x_first, x8_first, cs_prev,
                              first_of_seq=False, vec_sub=ht_, toff=ht_,
                              twid=ht_)
                    continue
                # split loads keep chunks landing ahead of the matmul
                # stream (a monolithic 2MB DMA starves the PE early on)
                xt, x8t = load_x(bi, n, n_dmas=4 if bi == 0 else 2)
                last_unit = (bi == b - 1) and (n == nu - 1)
                emit_unit(bi, n, xt, x8t, cs_prev, first_of_seq=(n == 0),
                          vec_sub=128 if last_unit else tt)

    nc.compile()
    return nc


_NC_CACHE = {}


def _get_nc():
    if "nc" not in _NC_CACHE:
        _NC_CACHE["nc"] = build_nc()
    return _NC_CACHE["nc"]


def make_in_maps(x, Wq, bq, Wk, bk, Wv, bv, Wg, bg, e=E, n_cores=N_CORES):
    xs = np.asarray(x, dtype=np.float32) * SX
    # X5[b, n, p, c, t] = SX*x[b, n*TT+t, c*128+p]
    xt = xs.reshape(B, NU, TT, KC, 128).transpose(0, 1, 4, 3, 2)
    X5 = np.ascontiguousarray(xt).astype(MM_NP)
    X8 = np.clip(np.ascontiguousarray(xt[:, :, :, :NX8, :]), -240, 240).astype(F8_NP)
    Ws = {"q": Wq, "k": Wk, "v": Wv, "g": Wg}
    bs = {"q": bq, "k": bk, "v": bv, "g": bg}
    in_maps = []
    for core in range(n_cores):
        sl = slice(core * e, (core + 1) * e)
        m = {"X5": X5, "X8": X8}
        for x_ in "qkvg":
            n8 = NF8[x_]
            # [p, m, c, e'] = W[sl][m*128+e', c*128+p] (x SW for fp8 chains)
            w = np.asarray(Ws[x_][sl, :], dtype=np.float32).T
            if n8:
                w = w * SW
            w = w.reshape(KC, 128, MH, 128).transpose(1, 2, 0, 3)
            m[f"W{x_}b"] = np.ascontiguousarray(w[:, :, n8:, :]).astype(MM_NP)
            if n8:
                m[f"W{x_}8"] = np.clip(
                    np.ascontiguousarray(w[:, :, :n8, :]), -240, 240
                ).astype(F8_NP)
            m[f"b{x_}"] = np.ascontiguousarray(np.asarray(bs[x_][sl], dtype=np.float32))
        in_maps.append(m)
    return in_maps


def gather_out(results, n_cores=N_CORES):
    # each core returns outT [B, E, S] f32; full out = [B, S, D] f32
    outs = [r["outT"] for r in results]
    full = np.concatenate(outs, axis=1)  # [B, D, S]
    return np.ascontiguousarray(full.transpose(0, 2, 1).astype(np.float32))


def kernel(x, Wq, bq, Wk, bk, Wv, bv, Wg, bg, **run_kwargs):
    nc = _get_nc()
    in_maps = make_in_maps(x, Wq, bq, Wk, bk, Wv, bv, Wg, bg)
    res = run_bass_kernel_spmd(
        nc, in_maps, core_ids=list(range(N_CORES)), **run_kwargs
    )
    out = gather_out(res.results)
    if run_kwargs:
        _NC_CACHE["last_result"] = res
    return out


# revision 12
# speedup vs baseline: 1.0088x; 1.0088x over previous
# Trainium2 Bass kernel for:
#   q = x @ Wq.T + bq ; k = x @ Wk.T + bk ; v = x @ Wv.T + bv
#   g = sigmoid(x @ Wg.T + bg)
#   out = q * cumsum(k*v, axis=seq) * g
#
# Sharding: tensor-parallel split of the 2048 output features across the 8
# cores (256 features each). All ops are per-feature except the d-contraction
# (each core uses the full x) and the cumsum along seq (handled fully on-core
# per (batch, feature)) -> zero cross-core communication.
#
# v3 over the bf16 baseline (912.5us):
#   - Part of the contraction runs as fp8(e4m3) DoubleRow matmuls: 14 of
#     g's 16 chunks and 2 of v's (numerics sim vs the jax reference —
#     which matched HW to 4 digits on two configs — puts max-err/scale
#     at 1.57e-2 vs the 2e-2 gate; more fp8 anywhere crosses 1.9e-2).
#     Warm DR matmuls stream 256 contraction rows in the same 216ns a
#     bf16 matmul needs for 128 rows (full 2x; LDW hidden).
#   - The fp8 and bf16 partial sums share one PSUM accumulator by
#     pre-scaling x by 32 and Wv/Wg by 4096 (exact powers of 2); the ACT
#     descales (1/32 for k,q; 2^-17 for v,g) in the existing bias-add.
#   - DR->bf16 mode switches cost ~220ns, so the v-DR and g-DR sections
#     are emitted back-to-back (chain order k,q,v,g with v's DR last and
#     g's DR first) -> one switch per m-group instead of two.
#   - fp32 downstream (k,v,q,g,kv,qg,out + out DMA) halves the rounding
#     floor (6.7e-3 -> 3.1e-3), buying the fp8 error budget.
#   - 110 dummy warmup matmuls on a zeroed tile keep the PE busy from the
#     end of the ~7.4us framework init so the HAM clock gate opens
#     (1.2 -> 2.4 GHz) during the DMA-bound prologue, not at t=26us.
#   - W packed m-major ([128, MH, KC, 128]) so the first chain only
#     gates on x(0,0) + Wk's m=0 half; halves stream in consumption
#     order; x8 rides the scalar SWDGE queue in parallel with the big
#     sync-queue stream.
#
# On-core layout is [e, t] (features on partitions, tokens on the free dim):
#   - linears:  psum[e,t] += W_chunk.T @ x_chunk   (fp32 accum)
#   - bias:     ACT activation Identity with per-partition bias + descale
#   - sigmoid:  ACT activation with per-partition bias + descale
#   - cumsum:   DVE tensor_tensor_scan along the free dim (fp32),
#               chained across token (sub)tiles via initial=prev[:, -1:]
#   - qg mul on the Pool engine, kv/out muls on DVE.
# The final unit is processed in 128-token sub-tiles to shorten the
# post-matmul drain chain.

from contextlib import ExitStack

import numpy as np
import ml_dtypes

import concourse.bass as bass  # noqa: F401  (bass types referenced via tile/bacc)
import concourse.tile as tile
from concourse import bacc, mybir
from concourse.bass_utils import run_bass_kernel_spmd

N_CORES = 8
B, S, D = 4, 4096, 2048
E = D // N_CORES  # 256 output features per core
TT = 512          # token tile (free dim of psum)
KC = D // 128     # contraction chunks
NU = S // TT      # token tiles per batch
MH = E // 128     # feature halves (psum groups per linear)
MM_DT = mybir.dt.bfloat16
MM_NP = ml_dtypes.bfloat16
F8_DT = mybir.dt.float8e4
F8_NP = ml_dtypes.float8_e4m3  # TRN fp8e4: max normal 240, matches after clip

# per-chain fp8 contraction chunks (each must be even; fp8 covers the FIRST
# nf8 chunks of that chain's contraction)
NF8 = {"q": 0, "k": 0, "v": 2, "g": 14}
NX8 = max(NF8.values())  # chunks of x kept in fp8
SX = 32.0                # x pre-scale (exact in bf16; uses e4m3 range)
SW = 4096.0              # W pre-scale for chains with fp8 chunks
N_WARM = 130             # dummy warmup matmuls before the real stream


def build_nc(b=B, s=S, d=D, e=E, tt=TT, n_cores=N_CORES):
    kc = KC
    nu = NU
    mh = MH
    f32 = mybir.dt.float32
    names = "qkvg"

    nc = bacc.Bacc(
        "TRN2", target_bir_lowering=False, debug=False, num_devices=n_cores
    )
    # x packed on host (pre-scaled by SX): X5[b, n, p, c, t] = SX*x[b, n*tt+t, c*128+p]
    X5 = nc.dram_tensor(
        "X5", [b, nu, 128, kc, tt], MM_DT, kind="ExternalInput"
    ).ap()
    # fp8 copy of x's first NX8 chunks (same SX scale)
    X8 = nc.dram_tensor(
        "X8", [b, nu, 128, NX8, tt], F8_DT, kind="ExternalInput"
    ).ap()
    # W packed on host, m-major: [p, m, c, e'] = W[core_sl][m*128+e', c*128+p]
    # chains with fp8 chunks ship W (x SW) as a fp8 part + a bf16 part
    Wb = {}
    W8 = {}
    for x_ in names:
        n8 = NF8[x_]
        Wb[x_] = nc.dram_tensor(
            f"W{x_}b", [128, mh, kc - n8, 128], MM_DT, kind="ExternalInput"
        ).ap()
        if n8:
            W8[x_] = nc.dram_tensor(
                f"W{x_}8", [128, mh, n8, 128], F8_DT, kind="ExternalInput"
            ).ap()
    bias = {
        x_: nc.dram_tensor(f"b{x_}", [e], f32, kind="ExternalInput").ap()
        for x_ in names
    }
    outT = nc.dram_tensor("outT", [b, e, s], f32, kind="ExternalOutput").ap()

    add = mybir.AluOpType.add
    bypass = mybir.AluOpType.bypass
    mult = mybir.AluOpType.mult
    sigmoid = mybir.ActivationFunctionType.Sigmoid
    identity = mybir.ActivationFunctionType.Identity
    dr = mybir.MatmulPerfMode.DoubleRow
    descale = {x_: 1.0 / (SX * (SW if NF8[x_] else 1.0)) for x_ in names}

    with tile.TileContext(nc) as tc, ExitStack() as ctx:
        wpool = ctx.enter_context(tc.tile_pool(name="w", bufs=1))
        cpool = ctx.enter_context(tc.tile_pool(name="const", bufs=1))
        xpool = ctx.enter_context(tc.tile_pool(name="x", bufs=3))
        x8pool = ctx.enter_context(tc.tile_pool(name="x8", bufs=3))
        ppool = ctx.enter_context(tc.tile_pool(name="psum", bufs=8, space="PSUM"))
        spool = ctx.enter_context(tc.tile_pool(name="work", bufs=5))
        opool = ctx.enter_context(tc.tile_pool(name="out", bufs=3))
        cspool = ctx.enter_context(tc.tile_pool(name="cs", bufs=6))

        # Biases via the gpsimd SWDGE queue (parallel with the sync stream):
        # [128, mh], col m = bias[m*128:(m+1)*128]
        b_sb = {}
        for x_ in names:
            t_ = cpool.tile([128, mh], f32, tag=f"b{x_}")
            nc.gpsimd.dma_start(out=t_, in_=bias[x_].rearrange("(m p) -> p m", p=128))
            b_sb[x_] = t_

        # PE warmup: dummy matmuls on a zeroed tile keep the PE's HAM
        # activity window busy while the prologue DMAs stream, so the
        # 1.2->2.4 GHz un-throttle fires before the real chains start.
        if N_WARM:
            dz = cpool.tile([128, 128], MM_DT, tag="warmz")
            nc.vector.memset(dz[:], 0.0)
            pd = ppool.tile([128, tt], f32, tag="ps")
            for _ in range(N_WARM):
                nc.tensor.matmul(
                    pd[:, :64], lhsT=dz[:], rhs=dz[:, :64], start=True, stop=True
                )

        wb_sb = {}
        w8_sb = {}
        for x_ in names:
            n8 = NF8[x_]
            t_ = wpool.tile([128, mh, kc - n8, 128], MM_DT, tag=f"w{x_}b")
            wb_sb[x_] = t_
            if n8:
                t8_ = wpool.tile([128, mh, n8, 128], F8_DT, tag=f"w{x_}8")
                w8_sb[x_] = t8_

        def load_x(bi, n, n_dmas=1, dual_queue=False):
            # bf16 unit tile (sync queue) + fp8 chunk tile (scalar queue);
            # n_dmas>1 splits the bf16 transfer so early chunks land (and
            # unblock matmuls) sooner; dual_queue alternates the splits
            # between both queues while the pipeline is still ramping
            t_ = xpool.tile([128, kc, tt], MM_DT, tag="xt")
            step = kc // n_dmas
            for i, c0 in enumerate(range(0, kc, step)):
                eng = nc.scalar if (dual_queue and i % 2 == 1) else nc.sync
                eng.dma_start(
                    out=t_[:, c0:c0 + step, :], in_=X5[bi][n][:, c0:c0 + step, :]
                )
            t8 = x8pool.tile([128, NX8, tt], F8_DT, tag="x8t")
            nc.scalar.dma_start(out=t8, in_=X8[bi][n])
            return t_, t8

        def load_w_half(x_, m):
            # one m-half of a chain's weights, in chain consumption order
            nc.sync.dma_start(out=wb_sb[x_][:, m], in_=Wb[x_][:, m])
            if NF8[x_]:
                nc.sync.dma_start(out=w8_sb[x_][:, m], in_=W8[x_][:, m])

        # Consumption-ordered prologue. x(0,0)'s chunk pairs alternate
        # between the sync and scalar DMA queues (two engines pulling in
        # parallel approach the HBM read rate, halving the time to the
        # first complete unit), with Wk-m0 quarters interleaved on sync.
        # Then x8(0,0) on scalar while sync streams the remaining m0
        # weight halves in chain order (q, v, g), then the m1 halves,
        # then units (0,1)/(0,2).
        x_first = xpool.tile([128, kc, tt], MM_DT, tag="xt")
        x8_first = x8pool.tile([128, NX8, tt], F8_DT, tag="x8t")
        for h in range(kc // 2):
            eng = nc.sync if h % 2 == 0 else nc.scalar
            eng.dma_start(
                out=x_first[:, 2 * h:2 * h + 2, :],
                in_=X5[0][0][:, 2 * h:2 * h + 2, :],
            )
            if h % 2 == 0 and h < 8:
                nc.sync.dma_start(
                    out=wb_sb["k"][:, 0, 4 * (h // 2):4 * (h // 2) + 4, :],
                    in_=Wb["k"][:, 0, 4 * (h // 2):4 * (h // 2) + 4, :],
                )
        nc.scalar.dma_start(out=x8_first, in_=X8[0][0])
        for x_ in "qvg":
            load_w_half(x_, 0)
        for x_ in "kqvg":
            load_w_half(x_, 1)

        def emit_unit(bi, n, xt, x8t, cs_prev, first_of_seq, vec_sub,
                      toff=0, twid=None):
            """Matmul chains for tokens [toff, toff+twid) of unit (bi, n);
            the ACT/DVE chain runs on vec_sub-wide psum slices (narrow for
            the final unit so the post-matmul drain is short). twid<tt is
            used for the DMA-gated first unit so chains start on a half
            tile's worth of data."""
            tw_ = twid if twid is not None else tt
            tsl = slice(toff, toff + tw_)
            # chain order k,q,v,g: k early for the kv/scan chain, and v's
            # trailing DR section lands adjacent to g's leading one so the
            # DR->bf16 mode switch is paid once per m-group
            ps = {}
            for m in range(mh):
                for x_ in "kq":
                    p_ = ppool.tile([128, tt], f32, tag="ps")
                    for c in range(kc):
                        nc.tensor.matmul(
                            p_[:, :tw_],
                            lhsT=wb_sb[x_][:, m, c, :],
                            rhs=xt[:, c, tsl],
                            start=(c == 0),
                            stop=(c == kc - 1),
                        )
                    ps[x_, m] = p_
                # v: bf16 chunks [nv8, kc) first, then DR pairs [0, nv8)
                nv8 = NF8["v"]
                p_ = ppool.tile([128, tt], f32, tag="ps")
                for i in range(kc - nv8):
                    nc.tensor.matmul(
                        p_[:, :tw_],
                        lhsT=wb_sb["v"][:, m, i, :],
                        rhs=xt[:, nv8 + i, tsl],
                        start=(i == 0),
                        stop=False,
                    )
                for j in range(nv8 // 2):
                    nc.tensor.matmul(
                        p_[:, :tw_],
                        lhsT=w8_sb["v"][:, m, 2 * j:2 * j + 2, :],
                        rhs=x8t[:, 2 * j:2 * j + 2, tsl],
                        start=False,
                        stop=(j == nv8 // 2 - 1),
                        perf_mode=dr,
                    )
                ps["v", m] = p_
                # g: DR pairs [0, ng8) first, then bf16 chunks [ng8, kc)
                ng8 = NF8["g"]
                p_ = ppool.tile([128, tt], f32, tag="ps")
                for j in range(ng8 // 2):
                    nc.tensor.matmul(
                        p_[:, :tw_],
                        lhsT=w8_sb["g"][:, m, 2 * j:2 * j + 2, :],
                        rhs=x8t[:, 2 * j:2 * j + 2, tsl],
                        start=(j == 0),
                        stop=False,
                        perf_mode=dr,
                    )
                for i in range(kc - ng8):
                    nc.tensor.matmul(
                        p_[:, :tw_],
                        lhsT=wb_sb["g"][:, m, i, :],
                        rhs=xt[:, ng8 + i, tsl],
                        start=False,
                        stop=(i == kc - ng8 - 1),
                    )
                ps["g", m] = p_

            for m in range(mh):
                # pass 1: k/v bias-adds (+descale), kv, scan per sub-slice
                # (two-pass so q/g ACT ops never block later slices' k/v in
                # the queue)
                cs_list = []
                for t0 in range(0, tw_, vec_sub):
                    tw = vec_sub
                    sl = slice(t0, t0 + tw)
                    k_sb = spool.tile([128, tw], f32, tag="k")
                    nc.scalar.activation(
                        k_sb[:], ps["k", m][:, sl], identity,
                        bias=b_sb["k"][:, m:m + 1], scale=descale["k"],
                    )
                    v_sb = spool.tile([128, tw], f32, tag="v")
                    nc.scalar.activation(
                        v_sb[:], ps["v", m][:, sl], identity,
                        bias=b_sb["v"][:, m:m + 1], scale=descale["v"],
                    )
                    kv = spool.tile([128, tw], f32, tag="kv")
                    nc.vector.tensor_tensor(kv[:], k_sb[:], v_sb[:], mult)
                    cs = cspool.tile([128, tw], f32, tag="cs")
                    init = (0.0 if first_of_seq and t0 == 0
                            else cs_prev[m][:, -1:])
                    nc.vector.tensor_tensor_scan(
                        cs[:], kv[:], kv[:], init, op0=add, op1=bypass
                    )
                    cs_prev[m] = cs
                    cs_list.append(cs)
                # pass 2: q/g bias-adds, qg, output — sub-slices write into
                # one full-width tile so each m issues a single out-DMA
                o_sb = opool.tile([128, tw_], f32, tag="o")
                q_list = []
                for t0 in range(0, tw_, vec_sub):
                    sl = slice(t0, t0 + vec_sub)
                    q_sb = spool.tile([128, vec_sub], f32, tag="q")
                    nc.scalar.activation(
                        q_sb[:], ps["q", m][:, sl], identity,
                        bias=b_sb["q"][:, m:m + 1], scale=descale["q"],
                    )
                    q_list.append(q_sb)
                for i, t0 in enumerate(range(0, tw_, vec_sub)):
                    sl = slice(t0, t0 + vec_sub)
                    g_sb = spool.tile([128, vec_sub], f32, tag="g")
                    nc.scalar.activation(
                        g_sb[:], ps["g", m][:, sl], sigmoid,
                        bias=b_sb["g"][:, m:m + 1], scale=descale["g"],
                    )
                    qg = spool.tile([128, vec_sub], f32, tag="qg")
                    nc.gpsimd.tensor_tensor(qg[:], q_list[i][:], g_sb[:], mult)
                    nc.vector.tensor_tensor(o_sb[:, sl], qg[:], cs_list[i][:], mult)
                t_base = n * tt + toff
                nc.sync.dma_start(
                    out=outT[bi][m * 128:(m + 1) * 128, t_base:t_base + tw_],
                    in_=o_sb[:],
                )

        for bi in range(b):
            cs_prev = [None] * mh
            for n in range(nu):
                if bi == 0 and n == 0:
                    xt, x8t = x_first, x8_first
                else:
                    # split loads keep chunks landing ahead of the matmul
                    # stream (a monolithic 2MB DMA starves the PE early
                    # on); both queues pull while the pipeline ramps
                    xt, x8t = load_x(bi, n, n_dmas=4 if bi == 0 else 2,
                                     dual_queue=(bi == 0 and n <= 2))
                last_unit = (bi == b - 1) and (n == nu - 1)
                emit_unit(bi, n, xt, x8t, cs_prev, first_of_seq=(n == 0),
                          vec_sub=128 if last_unit else tt)

    nc.compile()
    return nc


_NC_CACHE = {}


def _get_nc():
    if "nc" not in _NC_CACHE:
        _NC_CACHE["nc"] = build_nc()
    return _NC_CACHE["nc"]


def make_in_maps(x, Wq, bq, Wk, bk, Wv, bv, Wg, bg, e=E, n_cores=N_CORES):
    xs = np.asarray(x, dtype=np.float32) * SX
    # X5[b, n, p, c, t] = SX*x[b, n*TT+t, c*128+p]
    xt = xs.reshape(B, NU, TT, KC, 128).transpose(0, 1, 4, 3, 2)
    X5 = np.ascontiguousarray(xt).astype(MM_NP)
    X8 = np.clip(np.ascontiguousarray(xt[:, :, :, :NX8, :]), -240, 240).astype(F8_NP)
    Ws = {"q": Wq, "k": Wk, "v": Wv, "g": Wg}
    bs = {"q": bq, "k": bk, "v": bv, "g": bg}
    in_maps = []
    for core in range(n_cores):
        sl = slice(core * e, (core + 1) * e)
        m = {"X5": X5, "X8": X8}
        for x_ in "qkvg":
            n8 = NF8[x_]
            # [p, m, c, e'] = W[sl][m*128+e', c*128+p] (x SW for fp8 chains)
            w = np.asarray(Ws[x_][sl, :], dtype=np.float32).T
            if n8:
                w = w * SW
            w = w.reshape(KC, 128, MH, 128).transpose(1, 2, 0, 3)
            m[f"W{x_}b"] = np.ascontiguousarray(w[:, :, n8:, :]).astype(MM_NP)
            if n8:
                m[f"W{x_}8"] = np.clip(
                    np.ascontiguousarray(w[:, :, :n8, :]), -240, 240
                ).astype(F8_NP)
            m[f"b{x_}"] = np.ascontiguousarray(np.asarray(bs[x_][sl], dtype=np.float32))
        in_maps.append(m)
    return in_maps


def gather_out(results, n_cores=N_CORES):
    # each core returns outT [B, E, S] f32; full out = [B, S, D] f32
    outs = [r["outT"] for r in results]
    full = np.concatenate(outs, axis=1)  # [B, D, S]
    return np.ascontiguousarray(full.transpose(0, 2, 1).astype(np.float32))


def kernel(x, Wq, bq, Wk, bk, Wv, bv, Wg, bg, **run_kwargs):
    nc = _get_nc()
    in_maps = make_in_maps(x, Wq, bq, Wk, bk, Wv, bv, Wg, bg)
    res = run_bass_kernel_spmd(
        nc, in_maps, core_ids=list(range(N_CORES)), **run_kwargs
    )
    out = gather_out(res.results)
    if run_kwargs:
        _NC_CACHE["last_result"] = res
    return out


# revision 14
# speedup vs baseline: 1.0090x; 1.0002x over previous
# Trainium2 Bass kernel for:
#   q = x @ Wq.T + bq ; k = x @ Wk.T + bk ; v = x @ Wv.T + bv
#   g = sigmoid(x @ Wg.T + bg)
#   out = q * cumsum(k*v, axis=seq) * g
#
# Sharding: tensor-parallel split of the 2048 output features across the 8
# cores (256 features each). All ops are per-feature except the d-contraction
# (each core uses the full x) and the cumsum along seq (handled fully on-core
# per (batch, feature)) -> zero cross-core communication.
#
# v3 over the bf16 baseline (912.5us):
#   - Part of the contraction runs as fp8(e4m3) DoubleRow matmuls: 14 of
#     g's 16 chunks and 2 of v's (numerics sim vs the jax reference —
#     which matched HW to 4 digits on two configs — puts max-err/scale
#     at 1.57e-2 vs the 2e-2 gate; more fp8 anywhere crosses 1.9e-2).
#     Warm DR matmuls stream 256 contraction rows in the same 216ns a
#     bf16 matmul needs for 128 rows (full 2x; LDW hidden).
#   - The fp8 and bf16 partial sums share one PSUM accumulator by
#     pre-scaling x by 32 and Wv/Wg by 4096 (exact powers of 2); the ACT
#     descales (1/32 for k,q; 2^-17 for v,g) in the existing bias-add.
#   - DR->bf16 mode switches cost ~220ns, so the v-DR and g-DR sections
#     are emitted back-to-back (chain order k,q,v,g with v's DR last and
#     g's DR first) -> one switch per m-group instead of two.
#   - fp32 downstream (k,v,q,g,kv,qg,out + out DMA) halves the rounding
#     floor (6.7e-3 -> 3.1e-3), buying the fp8 error budget.
#   - 110 dummy warmup matmuls on a zeroed tile keep the PE busy from the
#     end of the ~7.4us framework init so the HAM clock gate opens
#     (1.2 -> 2.4 GHz) during the DMA-bound prologue, not at t=26us.
#   - W packed m-major ([128, MH, KC, 128]) so the first chain only
#     gates on x(0,0) + Wk's m=0 half; halves stream in consumption
#     order; x8 rides the scalar SWDGE queue in parallel with the big
#     sync-queue stream.
#
# On-core layout is [e, t] (features on partitions, tokens on the free dim):
#   - linears:  psum[e,t] += W_chunk.T @ x_chunk   (fp32 accum)
#   - bias:     ACT activation Identity with per-partition bias + descale
#   - sigmoid:  ACT activation with per-partition bias + descale
#   - cumsum:   DVE tensor_tensor_scan along the free dim (fp32),
#               chained across token (sub)tiles via initial=prev[:, -1:]
#   - qg mul on the Pool engine, kv/out muls on DVE.
# The final unit is processed in 128-token sub-tiles to shorten the
# post-matmul drain chain.

from contextlib import ExitStack

import numpy as np
import ml_dtypes

import concourse.bass as bass  # noqa: F401  (bass types referenced via tile/bacc)
import concourse.tile as tile
from concourse import bacc, mybir
from concourse.bass_utils import run_bass_kernel_spmd

N_CORES = 8
B, S, D = 4, 4096, 2048
E = D // N_CORES  # 256 output features per core
TT = 512          # token tile (free dim of psum)
KC = D // 128     # contraction chunks
NU = S // TT      # token tiles per batch
MH = E // 128     # feature halves (psum groups per linear)
MM_DT = mybir.dt.bfloat16
MM_NP = ml_dtypes.bfloat16
F8_DT = mybir.dt.float8e4
F8_NP = ml_dtypes.float8_e4m3  # TRN fp8e4: max normal 240, matches after clip

# per-chain fp8 contraction chunks (each must be even; fp8 covers the FIRST
# nf8 chunks of that chain's contraction)
NF8 = {"q": 0, "k": 0, "v": 2, "g": 14}
NX8 = max(NF8.values())  # chunks of x kept in fp8
SX = 32.0                # x pre-scale (exact in bf16; uses e4m3 range)
SW = 4096.0              # W pre-scale for chains with fp8 chunks
N_WARM = 160             # dummy warmup matmuls before the real stream


def build_nc(b=B, s=S, d=D, e=E, tt=TT, n_cores=N_CORES):
    kc = KC
    nu = NU
    mh = MH
    f32 = mybir.dt.float32
    names = "qkvg"

    nc = bacc.Bacc(
        "TRN2", target_bir_lowering=False, debug=False, num_devices=n_cores
    )
    # x packed on host (pre-scaled by SX): X5[b, n, p, c, t] = SX*x[b, n*tt+t, c*128+p]
    X5 = nc.dram_tensor(
        "X5", [b, nu, 128, kc, tt], MM_DT, kind="ExternalInput"
    ).ap()
    # fp8 copy of x's first NX8 chunks (same SX scale)
    X8 = nc.dram_tensor(
        "X8", [b, nu, 128, NX8, tt], F8_DT, kind="ExternalInput"
    ).ap()
    # W packed on host, m-major: [p, m, c, e'] = W[core_sl][m*128+e', c*128+p]
    # chains with fp8 chunks ship W (x SW) as a fp8 part + a bf16 part
    Wb = {}
    W8 = {}
    for x_ in names:
        n8 = NF8[x_]
        Wb[x_] = nc.dram_tensor(
            f"W{x_}b", [128, mh, kc - n8, 128], MM_DT, kind="ExternalInput"
        ).ap()
        if n8:
            W8[x_] = nc.dram_tensor(
                f"W{x_}8", [128, mh, n8, 128], F8_DT, kind="ExternalInput"
            ).ap()
    bias = {
        x_: nc.dram_tensor(f"b{x_}", [e], f32, kind="ExternalInput").ap()
        for x_ in names
    }
    outT = nc.dram_tensor("outT", [b, e, s], f32, kind="ExternalOutput").ap()

    add = mybir.AluOpType.add
    bypass = mybir.AluOpType.bypass
    mult = mybir.AluOpType.mult
    sigmoid = mybir.ActivationFunctionType.Sigmoid
    identity = mybir.ActivationFunctionType.Identity
    dr = mybir.MatmulPerfMode.DoubleRow
    descale = {x_: 1.0 / (SX * (SW if NF8[x_] else 1.0)) for x_ in names}

    with tile.TileContext(nc) as tc, ExitStack() as ctx:
        wpool = ctx.enter_context(tc.tile_pool(name="w", bufs=1))
        cpool = ctx.enter_context(tc.tile_pool(name="const", bufs=1))
        xpool = ctx.enter_context(tc.tile_pool(name="x", bufs=3))
        x8pool = ctx.enter_context(tc.tile_pool(name="x8", bufs=3))
        ppool = ctx.enter_context(tc.tile_pool(name="psum", bufs=8, space="PSUM"))
        spool = ctx.enter_context(tc.tile_pool(name="work", bufs=5))
        opool = ctx.enter_context(tc.tile_pool(name="out", bufs=3))
        cspool = ctx.enter_context(tc.tile_pool(name="cs", bufs=6))

        # Biases via the gpsimd SWDGE queue (parallel with the sync stream):
        # [128, mh], col m = bias[m*128:(m+1)*128]
        b_sb = {}
        for x_ in names:
            t_ = cpool.tile([128, mh], f32, tag=f"b{x_}")
            nc.gpsimd.dma_start(out=t_, in_=bias[x_].rearrange("(m p) -> p m", p=128))
            b_sb[x_] = t_

        # PE warmup: dummy matmuls on a zeroed tile keep the PE's HAM
        # activity window busy while the prologue DMAs stream, so the
        # 1.2->2.4 GHz un-throttle fires before the real chains start.
        if N_WARM:
            dz = cpool.tile([128, 128], MM_DT, tag="warmz")
            nc.vector.memset(dz[:], 0.0)
            pd = ppool.tile([128, tt], f32, tag="ps")
            for _ in range(N_WARM):
                nc.tensor.matmul(
                    pd[:, :64], lhsT=dz[:], rhs=dz[:, :64], start=True, stop=True
                )

        wb_sb = {}
        w8_sb = {}
        for x_ in names:
            n8 = NF8[x_]
            t_ = wpool.tile([128, mh, kc - n8, 128], MM_DT, tag=f"w{x_}b")
            wb_sb[x_] = t_
            if n8:
                t8_ = wpool.tile([128, mh, n8, 128], F8_DT, tag=f"w{x_}8")
                w8_sb[x_] = t8_

        def load_x(bi, n, n_dmas=1, dual_queue=False):
            # bf16 unit tile (sync queue) + fp8 chunk tile (scalar queue);
            # n_dmas>1 splits the bf16 transfer so early chunks land (and
            # unblock matmuls) sooner; dual_queue alternates the splits
            # between both queues while the pipeline is still ramping
            t_ = xpool.tile([128, kc, tt], MM_DT, tag="xt")
            step = kc // n_dmas
            for i, c0 in enumerate(range(0, kc, step)):
                eng = nc.scalar if (dual_queue and i % 2 == 1) else nc.sync
                eng.dma_start(
                    out=t_[:, c0:c0 + step, :], in_=X5[bi][n][:, c0:c0 + step, :]
                )
            t8 = x8pool.tile([128, NX8, tt], F8_DT, tag="x8t")
            nc.scalar.dma_start(out=t8, in_=X8[bi][n])
            return t_, t8

        def load_w_half(x_, m):
            # one m-half of a chain's weights, in chain consumption order
            nc.sync.dma_start(out=wb_sb[x_][:, m], in_=Wb[x_][:, m])
            if NF8[x_]:
                nc.sync.dma_start(out=w8_sb[x_][:, m], in_=W8[x_][:, m])

        # Consumption-ordered prologue. x(0,0)'s chunk pairs alternate
        # between the sync and scalar DMA queues (two engines pulling in
        # parallel approach the HBM read rate, halving the time to the
        # first complete unit), with Wk-m0 quarters interleaved on sync.
        # Then x8(0,0) on scalar while sync streams the remaining m0
        # weight halves in chain order (q, v, g), then the m1 halves,
        # then units (0,1)/(0,2).
        x_first = xpool.tile([128, kc, tt], MM_DT, tag="xt")
        x8_first = x8pool.tile([128, NX8, tt], F8_DT, tag="x8t")
        for h in range(kc // 2):
            eng = nc.sync if h % 2 == 0 else nc.scalar
            eng.dma_start(
                out=x_first[:, 2 * h:2 * h + 2, :],
                in_=X5[0][0][:, 2 * h:2 * h + 2, :],
            )
            if h % 2 == 0 and h < 8:
                nc.sync.dma_start(
                    out=wb_sb["k"][:, 0, 4 * (h // 2):4 * (h // 2) + 4, :],
                    in_=Wb["k"][:, 0, 4 * (h // 2):4 * (h // 2) + 4, :],
                )
        # scalar queue: the small m0 W parts for v/g, then x8(0,0) —
        # in parallel with sync's Wq/Wvb m0 halves so the first unit's
        # later chains aren't gated on one queue draining ~3MB
        nc.scalar.dma_start(out=w8_sb["v"][:, 0], in_=W8["v"][:, 0])
        nc.scalar.dma_start(out=w8_sb["g"][:, 0], in_=W8["g"][:, 0])
        nc.scalar.dma_start(out=wb_sb["g"][:, 0], in_=Wb["g"][:, 0])
        nc.scalar.dma_start(out=x8_first, in_=X8[0][0])
        nc.sync.dma_start(out=wb_sb["q"][:, 0], in_=Wb["q"][:, 0])
        nc.sync.dma_start(out=wb_sb["v"][:, 0], in_=Wb["v"][:, 0])
        for x_ in "kqvg":
            load_w_half(x_, 1)

        def emit_unit(bi, n, xt, x8t, cs_prev, first_of_seq, vec_sub,
                      toff=0, twid=None):
            """Matmul chains for tokens [toff, toff+twid) of unit (bi, n);
            the ACT/DVE chain runs on vec_sub-wide psum slices (narrow for
            the final unit so the post-matmul drain is short). twid<tt is
            used for the DMA-gated first unit so chains start on a half
            tile's worth of data."""
            tw_ = twid if twid is not None else tt
            tsl = slice(toff, toff + tw_)
            # chain order k,q,v,g: k early for the kv/scan chain, and v's
            # trailing DR section lands adjacent to g's leading one so the
            # DR->bf16 mode switch is paid once per m-group
            ps = {}
            for m in range(mh):
                for x_ in "kq":
                    p_ = ppool.tile([128, tt], f32, tag="ps")
                    for c in range(kc):
                        nc.tensor.matmul(
                            p_[:, :tw_],
                            lhsT=wb_sb[x_][:, m, c, :],
                            rhs=xt[:, c, tsl],
                            start=(c == 0),
                            stop=(c == kc - 1),
                        )
                    ps[x_, m] = p_
                # v: bf16 chunks [nv8, kc) first, then DR pairs [0, nv8)
                nv8 = NF8["v"]
                p_ = ppool.tile([128, tt], f32, tag="ps")
                for i in range(kc - nv8):
                    nc.tensor.matmul(
                        p_[:, :tw_],
                        lhsT=wb_sb["v"][:, m, i, :],
                        rhs=xt[:, nv8 + i, tsl],
                        start=(i == 0),
                        stop=False,
                    )
                for j in range(nv8 // 2):
                    nc.tensor.matmul(
                        p_[:, :tw_],
                        lhsT=w8_sb["v"][:, m, 2 * j:2 * j + 2, :],
                        rhs=x8t[:, 2 * j:2 * j + 2, tsl],
                        start=False,
                        stop=(j == nv8 // 2 - 1),
                        perf_mode=dr,
                    )
                ps["v", m] = p_
                # g: DR pairs [0, ng8) first, then bf16 chunks [ng8, kc)
                ng8 = NF8["g"]
                p_ = ppool.tile([128, tt], f32, tag="ps")
                for j in range(ng8 // 2):
                    nc.tensor.matmul(
                        p_[:, :tw_],
                        lhsT=w8_sb["g"][:, m, 2 * j:2 * j + 2, :],
                        rhs=x8t[:, 2 * j:2 * j + 2, tsl],
                        start=(j == 0),
                        stop=False,
                        perf_mode=dr,
                    )
                for i in range(kc - ng8):
                    nc.tensor.matmul(
                        p_[:, :tw_],
                        lhsT=wb_sb["g"][:, m, i, :],
                        rhs=xt[:, ng8 + i, tsl],
                        start=False,
                        stop=(i == kc - ng8 - 1),
                    )
                ps["g", m] = p_

            for m in range(mh):
                # pass 1: k/v bias-adds (+descale), kv, scan per sub-slice
                # (two-pass so q/g ACT ops never block later slices' k/v in
                # the queue)
                cs_list = []
                for t0 in range(0, tw_, vec_sub):
                    tw = vec_sub
                    sl = slice(t0, t0 + tw)
                    k_sb = spool.tile([128, tw], f32, tag="k")
                    nc.scalar.activation(
                        k_sb[:], ps["k", m][:, sl], identity,
                        bias=b_sb["k"][:, m:m + 1], scale=descale["k"],
                    )
                    v_sb = spool.tile([128, tw], f32, tag="v")
                    nc.scalar.activation(
                        v_sb[:], ps["v", m][:, sl], identity,
                        bias=b_sb["v"][:, m:m + 1], scale=descale["v"],
                    )
                    kv = spool.tile([128, tw], f32, tag="kv")
                    nc.vector.tensor_tensor(kv[:], k_sb[:], v_sb[:], mult)
                    cs = cspool.tile([128, tw], f32, tag="cs")
                    init = (0.0 if first_of_seq and t0 == 0
                            else cs_prev[m][:, -1:])
                    nc.vector.tensor_tensor_scan(
                        cs[:], kv[:], kv[:], init, op0=add, op1=bypass
                    )
                    cs_prev[m] = cs
                    cs_list.append(cs)
                # pass 2: q/g bias-adds, qg, output — sub-slices write into
                # one full-width tile so each m issues a single out-DMA
                o_sb = opool.tile([128, tw_], f32, tag="o")
                q_list = []
                for t0 in range(0, tw_, vec_sub):
                    sl = slice(t0, t0 + vec_sub)
                    q_sb = spool.tile([128, vec_sub], f32, tag="q")
                    nc.scalar.activation(
                        q_sb[:], ps["q", m][:, sl], identity,
                        bias=b_sb["q"][:, m:m + 1], scale=descale["q"],
                    )
                    q_list.append(q_sb)
                for i, t0 in enumerate(range(0, tw_, vec_sub)):
                    sl = slice(t0, t0 + vec_sub)
                    g_sb = spool.tile([128, vec_sub], f32, tag="g")
                    nc.scalar.activation(
                        g_sb[:], ps["g", m][:, sl], sigmoid,
                        bias=b_sb["g"][:, m:m + 1], scale=descale["g"],
                    )
                    qg = spool.tile([128, vec_sub], f32, tag="qg")
                    nc.gpsimd.tensor_tensor(qg[:], q_list[i][:], g_sb[:], mult)
                    nc.vector.tensor_tensor(o_sb[:, sl], qg[:], cs_list[i][:], mult)
                t_base = n * tt + toff
                nc.sync.dma_start(
                    out=outT[bi][m * 128:(m + 1) * 128, t_base:t_base + tw_],
                    in_=o_sb[:],
                )

        for bi in range(b):
            cs_prev = [None] * mh
            for n in range(nu):
                if bi == 0 and n == 0:
                    xt, x8t = x_first, x8_first
                else:
                    # split loads keep chunks landing ahead of the matmul
                    # stream (a monolithic 2MB DMA starves the PE early
                    # on); both queues pull while the pipeline ramps
                    xt, x8t = load_x(bi, n, n_dmas=4 if bi == 0 else 2,
                                     dual_queue=(bi == 0 and n <= 2))
                last_unit = (bi == b - 1) and (n == nu - 1)
                emit_unit(bi, n, xt, x8t, cs_prev, first_of_seq=(n == 0),
                          vec_sub=128 if last_unit else tt)

    nc.compile()
    return nc


_NC_CACHE = {}


def _get_nc():
    if "nc" not in _NC_CACHE:
        _NC_CACHE["nc"] = build_nc()
    return _NC_CACHE["nc"]


def make_in_maps(x, Wq, bq, Wk, bk, Wv, bv, Wg, bg, e=E, n_cores=N_CORES):
    xs = np.asarray(x, dtype=np.float32) * SX
    # X5[b, n, p, c, t] = SX*x[b, n*TT+t, c*128+p]
    xt = xs.reshape(B, NU, TT, KC, 128).transpose(0, 1, 4, 3, 2)
    X5 = np.ascontiguousarray(xt).astype(MM_NP)
    X8 = np.clip(np.ascontiguousarray(xt[:, :, :, :NX8, :]), -240, 240).astype(F8_NP)
    Ws = {"q": Wq, "k": Wk, "v": Wv, "g": Wg}
    bs = {"q": bq, "k": bk, "v": bv, "g": bg}
    in_maps = []
    for core in range(n_cores):
        sl = slice(core * e, (core + 1) * e)
        m = {"X5": X5, "X8": X8}
        for x_ in "qkvg":
            n8 = NF8[x_]
            # [p, m, c, e'] = W[sl][m*128+e', c*128+p] (x SW for fp8 chains)
            w = np.asarray(Ws[x_][sl, :], dtype=np.float32).T
            if n8:
                w = w * SW
            w = w.reshape(KC, 128, MH, 128).transpose(1, 2, 0, 3)
            m[f"W{x_}b"] = np.ascontiguousarray(w[:, :, n8:, :]).astype(MM_NP)
            if n8:
                m[f"W{x_}8"] = np.clip(
                    np.ascontiguousarray(w[:, :, :n8, :]), -240, 240
                ).astype(F8_NP)
            m[f"b{x_}"] = np.ascontiguousarray(np.asarray(bs[x_][sl], dtype=np.float32))
        in_maps.append(m)
    return in_maps


def gather_out(results, n_cores=N_CORES):
    # each core returns outT [B, E, S] f32; full out = [B, S, D] f32
    outs = [r["outT"] for r in results]
    full = np.concatenate(outs, axis=1)  # [B, D, S]
    return np.ascontiguousarray(full.transpose(0, 2, 1).astype(np.float32))


def kernel(x, Wq, bq, Wk, bk, Wv, bv, Wg, bg, **run_kwargs):
    nc = _get_nc()
    in_maps = make_in_maps(x, Wq, bq, Wk, bk, Wv, bv, Wg, bg)
    res = run_bass_kernel_spmd(
        nc, in_maps, core_ids=list(range(N_CORES)), **run_kwargs
    )
    out = gather_out(res.results)
    if run_kwargs:
        _NC_CACHE["last_result"] = res
    return out
